# revision 5
# baseline (speedup 1.0000x reference)
"""GIN 2-layer message-passing network on 8 Trainium2 NeuronCores.

v2 strategy (dst-partitioned, per the sharding hint):
  - Nodes split into 8 chunks of N/8; core c owns chunk c and all edges
    whose destination lands in it.
  - Edges are grouped by (64-wide dst group, source quarter) and padded to
    128-slot chunks (cross-core max so one SPMD NEFF serves all cores).
    Gathered source rows (bf16) are scatter-added on the tensor engine
    via one-hot matmuls:  agg[feat, d] += G[e, feat].T @ OH[e, d<64>].
    64-wide one-hot columns halve the DVE is_equal work vs 128-wide.
  - The GIN self term (1+eps)*h_i is realized by initializing each dst
    tile's PSUM accumulation with an identity matmul of the tile's own
    feature rows (no self-edges in the gather).
  - MLP runs in transposed land ([feat, nodes]); per 128-dst tile:
    h = relu(w.T @ aggT + b) via PE matmul + ACT relu-with-bias.
  - Between layers, per-core h chunks are exchanged with 4 AllGathers
    (one per quarter, fired as soon as their rows are done) into Shared
    DRAM tensors so layer-2 gathers can use int16 indices.
  - log_softmax of the final [40, nodes] tile after a PE transpose.

All per-core variability lives in the data (gather indices / dst-local
arrays, padded to a per-group max across cores) so a single SPMD NEFF
serves all 8 cores.
"""

import os
import sys

sys.path.insert(0, "/opt/trn_rl_repo")
sys.path.insert(0, "/opt/trn_rl_repo/concourse")
os.environ.setdefault("TRN_TYPE", "TRN2")

import numpy as np
import ml_dtypes

BF16 = ml_dtypes.bfloat16

NCORES = 8
W = 64                    # dst-group width (one-hot column count)


class Cfg:
    def __init__(self, n, feat, hid, cls, tiles_per_batch=5):
        assert n % (NCORES * 4) == 0
        self.N = n
        self.F = feat
        self.H = hid
        self.CLS = cls
        self.NPC = n // NCORES          # nodes per core
        self.QROWS = self.NPC // 4      # rows per quarter per core
        self.SRCROWS = self.QROWS * NCORES  # rows per gather-source tensor
        self.NT = -(-self.NPC // 128)   # 128-dst tiles per core
        self.NG = self.NT * 2           # 64-wide dst groups per core
        self.last_rows = self.NPC - (self.NT - 1) * 128
        self.B = tiles_per_batch


FULL = Cfg(100000, 128, 128, 40, tiles_per_batch=5)


def _prep_graph(edge_index, cfg):
    """Host-side sharding. Groups edges by (dst 64-group, src quarter),
    pads each group to 128-slot chunks with the max count across cores.

    Returns (schedule, per-core gidx wraps, per-core dstloc arrays)."""
    N, NPC, QROWS, NG = cfg.N, cfg.NPC, cfg.QROWS, cfg.NG
    src = np.asarray(edge_index[0], dtype=np.int64)
    dst = np.asarray(edge_index[1], dtype=np.int64)

    core = dst // NPC
    per_core = []
    counts = np.zeros((NCORES, NG * 4), np.int64)
    for c in range(NCORES):
        m = core == c
        s = src[m]
        dloc = dst[m] - c * NPC
        g = dloc // W
        q = (s % NPC) // QROWS
        gid = g * 4 + q
        gidxv = (s // NPC) * QROWS + (s % QROWS)
        dstin = dloc % W
        counts[c] = np.bincount(gid, minlength=NG * 4)
        per_core.append((gid, gidxv.astype(np.int32), dstin.astype(np.int32)))

    cmax = counts.max(axis=0)
    C = -(-cmax // 128)                 # chunks per (group, quarter)
    slots = C * 128
    B = cfg.B
    batches = [list(range(b, min(b + B, cfg.NT))) for b in range(0, cfg.NT, B)]
    off = 0
    slot_off = np.zeros(NG * 4, np.int64)
    call_slots, call_off = [], []
    for tiles in batches:
        cs, co = [], []
        groups = [2 * t + h for t in tiles for h in range(2)]
        for q in range(4):
            co.append(off)
            s0 = off
            for g in groups:
                slot_off[g * 4 + q] = off
                off += slots[g * 4 + q]
            cs.append(off - s0)
        call_slots.append(cs)
        call_off.append(co)
    tot = off
    assert tot % 128 == 0

    gidx_all, dstloc_all = [], []
    for c in range(NCORES):
        gid, gidxv, dstin = per_core[c]
        order = np.argsort(gid, kind="stable")
        gs = gid[order]
        cnt = counts[c]
        starts = np.zeros(NG * 4, np.int64)
        np.cumsum(cnt[:-1], out=starts[1:])
        rank = np.arange(len(gs)) - starts[gs]
        slot = slot_off[gs] + rank
        gflat = np.zeros(tot, np.int16)
        dflat = np.full(tot, 200.0, np.float32)
        gflat[slot] = gidxv[order].astype(np.int16)
        dflat[slot] = dstin[order]
        # wrap for dma_gather: [p, col] = gflat[col*16 + p%16], replicated x8
        gwr = np.tile(gflat.reshape(tot // 16, 16).T, (8, 1)).copy()
        dloc = dflat.reshape(tot // 128, 128).T.astype(BF16).copy()
        gidx_all.append(gwr)
        dstloc_all.append(dloc)

    sched = dict(C=C, slots=slots, batches=batches, call_slots=call_slots,
                 call_off=call_off, slot_off=slot_off, tot=tot)
    return sched, gidx_all, dstloc_all


def _perm_rows(x, cfg):
    """x [N, F] -> 4 arrays [SRCROWS, F]; source s holds global row
    g = r*NPC + s*QROWS + u at position r*QROWS + u."""
    N, NPC, QROWS = cfg.N, cfg.NPC, cfg.QROWS
    g = np.arange(N)
    s = (g % NPC) // QROWS
    pos = (g // NPC) * QROWS + (g % QROWS)
    out = []
    for si in range(4):
        m = s == si
        a = np.empty((cfg.SRCROWS, x.shape[1]), x.dtype)
        a[pos[m]] = x[m]
        out.append(a)
    return out


def _build_nc(cfg, sched, eps1, eps2):
    from concourse import mybir
    import concourse.bacc as bacc
    import concourse.tile as tile

    F, H, CLS, NT, NPC = cfg.F, cfg.H, cfg.CLS, cfg.NT, cfg.NPC
    C = sched["C"]
    batches = sched["batches"]
    call_slots = sched["call_slots"]
    call_off = sched["call_off"]
    tot = sched["tot"]
    f32 = mybir.dt.float32
    bf16 = mybir.dt.bfloat16
    AT = mybir.ActivationFunctionType
    OP = mybir.AluOpType

    assert eps1 == 0.0 and eps2 == 0.0, "nonzero eps not implemented"

    nc = bacc.Bacc("TRN2", target_bir_lowering=False, debug=False,
                   num_devices=NCORES)

    xq = [nc.dram_tensor(f"xq{q}", [cfg.SRCROWS, F], bf16, kind="ExternalInput")
          for q in range(4)]
    xown_t = nc.dram_tensor("xown", [NT * 128, F], bf16, kind="ExternalInput")
    w1_t = nc.dram_tensor("w1", [F, H], f32, kind="ExternalInput")
    w2_t = nc.dram_tensor("w2", [H, H], f32, kind="ExternalInput")
    w3_t = nc.dram_tensor("w3", [H, H], f32, kind="ExternalInput")
    w4_t = nc.dram_tensor("w4", [H, CLS], f32, kind="ExternalInput")
    b1_t = nc.dram_tensor("b1", [H, 1], f32, kind="ExternalInput")
    b2_t = nc.dram_tensor("b2", [H, 1], f32, kind="ExternalInput")
    b3_t = nc.dram_tensor("b3", [H, 1], f32, kind="ExternalInput")
    b4_t = nc.dram_tensor("b4", [CLS, 1], f32, kind="ExternalInput")
    iota_t = nc.dram_tensor("iota", [128, W], bf16, kind="ExternalInput")
    identb_t = nc.dram_tensor("identb", [128, 128], bf16, kind="ExternalInput")
    ident_t = nc.dram_tensor("ident", [128, 128], f32, kind="ExternalInput")
    gidx_t = nc.dram_tensor("gidx", [128, tot // 16], mybir.dt.int16,
                            kind="ExternalInput")
    dstloc_t = nc.dram_tensor("dstloc", [128, tot // 128], bf16,
                              kind="ExternalInput")
    out_t = nc.dram_tensor("out", [NPC, CLS], f32, kind="ExternalOutput")

    maxS = max(max(cs) for cs in call_slots)

    with tile.TileContext(nc) as tc:
        with tc.tile_pool(name="const", bufs=1) as cp, \
             tc.tile_pool(name="gp", bufs=6) as gp, \
             tc.tile_pool(name="ohp", bufs=6) as ohp, \
             tc.tile_pool(name="idxp", bufs=3) as idxp, \
             tc.tile_pool(name="dlp", bufs=3) as dlp, \
             tc.tile_pool(name="ownp", bufs=6) as ownp, \
             tc.tile_pool(name="work", bufs=4) as wp, \
             tc.tile_pool(name="small", bufs=6) as sp, \
             tc.tile_pool(name="aggps", bufs=4, space="PSUM") as aggps, \
             tc.tile_pool(name="mmps", bufs=4, space="PSUM") as mmps, \
             tc.tile_pool(name="dram", bufs=1, space="DRAM") as dp, \
             tc.tile_pool(name="shdram", bufs=1, space="DRAM") as shp:

            w1 = cp.tile([F, H], f32); nc.sync.dma_start(w1[:], w1_t.ap())
            w2 = cp.tile([H, H], f32); nc.sync.dma_start(w2[:], w2_t.ap())
            w3 = cp.tile([H, H], f32); nc.sync.dma_start(w3[:], w3_t.ap())
            w4 = cp.tile([H, CLS], f32); nc.sync.dma_start(w4[:], w4_t.ap())
            b1 = cp.tile([H, 1], f32); nc.sync.dma_start(b1[:], b1_t.ap())
            b2 = cp.tile([H, 1], f32); nc.sync.dma_start(b2[:], b2_t.ap())
            b3 = cp.tile([H, 1], f32); nc.sync.dma_start(b3[:], b3_t.ap())
            b4 = cp.tile([CLS, 1], f32); nc.sync.dma_start(b4[:], b4_t.ap())
            iota = cp.tile([128, W], bf16); nc.sync.dma_start(iota[:], iota_t.ap())
            identb = cp.tile([128, 128], bf16)
            nc.sync.dma_start(identb[:], identb_t.ap())
            ident = cp.tile([128, 128], f32); nc.sync.dma_start(ident[:], ident_t.ap())
            stash = cp.tile([128, NT * 128], bf16)

            h_own = dp.tile([NPC, H], bf16)
            h_ag = [shp.tile([cfg.SRCROWS, H], bf16, addr_space="Shared",
                             name=f"h_ag{s}")
                    for s in range(4)]

            def layer(sources, is_first, batch_limit=None, skip_mlp=False):
                ag_next = [0]

                for b, tiles in enumerate(batches):
                    if batch_limit is not None and b >= batch_limit:
                        break
                    base = call_off[b][0]
                    bslots = call_off[b][3] + call_slots[b][3] - base
                    # batched index loads (gidx always; dstloc both layers)
                    gi = idxp.tile([128, maxS * 4 // 16], mybir.dt.int16,
                                   tag="idx")
                    nc.sync.dma_start(
                        gi[:, : bslots // 16],
                        gidx_t.ap()[:, base // 16:(base + bslots) // 16])
                    dl = dlp.tile([128, maxS * 4 // 128], bf16, tag="dl")
                    nc.sync.dma_start(
                        dl[:, : bslots // 128],
                        dstloc_t.ap()[:, base // 128:(base + bslots) // 128])

                    G, OH = [], []
                    for q in range(4):
                        S = call_slots[b][q]
                        if S == 0:
                            G.append(None); OH.append(None)
                            continue
                        o = call_off[b][q]
                        g = gp.tile([128, maxS // 128, 128], bf16, tag="g")
                        nc.gpsimd.dma_gather(
                            g[:, : S // 128, :], sources[q],
                            gi[:, (o - base) // 16:(o - base + S) // 16],
                            S, S, F, single_packet=False)
                        oh = ohp.tile([128, maxS // 128, W], bf16, tag="oh")
                        nc.vector.tensor_tensor(
                            out=oh[:, : S // 128, :],
                            in0=iota[:].unsqueeze(1).broadcast_to(
                                [128, S // 128, W]),
                            in1=dl[:, (o - base) // 128:(o - base + S) // 128]
                                .unsqueeze(2).broadcast_to([128, S // 128, W]),
                            op=OP.is_equal)
                        G.append(g); OH.append(oh)

                    pos = [0, 0, 0, 0]
                    for t in tiles:
                        # chunks per (half, quarter) for this tile
                        nch = [[int(C[(2 * t + h) * 4 + q]) for q in range(4)]
                               for h in range(2)]
                        tot_ch = sum(sum(r) for r in nch)
                        agg = aggps.tile([128, 128], f32, tag="agg")
                        own = None
                        if is_first:
                            own = ownp.tile([128, 128], bf16, tag="own")
                            nc.sync.dma_start(
                                own[:], xown_t.ap()[t * 128:(t + 1) * 128, :])
                            own_ap = own[:]
                        else:
                            own_ap = stash[:, t * 128:(t + 1) * 128]
                        # self term: agg = own.T (identity matmul opens group)
                        nc.tensor.matmul(out=agg[:], lhsT=own_ap, rhs=identb[:],
                                         start=True, stop=(tot_ch == 0))
                        k = 0
                        for h in range(2):
                            for q in range(4):
                                if nch[h][q] == 0:
                                    continue
                                for j in range(nch[h][q]):
                                    col = pos[q] + (0 if h == 0 else nch[0][q]) + j
                                    k += 1
                                    nc.tensor.matmul(
                                        out=agg[:, h * W:(h + 1) * W],
                                        lhsT=G[q][:, col, :],
                                        rhs=OH[q][:, col, :],
                                        start=False,
                                        stop=(k == tot_ch),
                                        skip_group_check=True)
                        for q in range(4):
                            pos[q] += nch[0][q] + nch[1][q]
                        rows = 128 if t < NT - 1 else cfg.last_rows

                        if skip_mlp:
                            continue
                        aggT = wp.tile([128, 128], f32, tag="aggT")
                        nc.scalar.activation(out=aggT[:], in_=agg[:], func=AT.Copy)

                        if is_first:
                            ps1 = mmps.tile([128, 128], f32, tag="mm")
                            nc.tensor.matmul(out=ps1[:], lhsT=w1[:], rhs=aggT[:],
                                             start=True, stop=True)
                            h1 = wp.tile([128, 128], f32, tag="h1")
                            nc.scalar.activation(out=h1[:], in_=ps1[:],
                                                 func=AT.Relu, bias=b1[:])
                            ps2 = mmps.tile([128, 128], f32, tag="mm")
                            nc.tensor.matmul(out=ps2[:], lhsT=w2[:], rhs=h1[:],
                                             start=True, stop=True)
                            h2 = wp.tile([128, 128], f32, tag="h2")
                            nc.scalar.activation(out=h2[:], in_=ps2[:],
                                                 func=AT.Relu, bias=b2[:])
                            # transpose back to [nodes, feat], cast bf16, stash
                            pst = mmps.tile([128, 128], f32, tag="mm")
                            nc.tensor.transpose(out=pst[:], in_=h2[:],
                                                identity=ident[:])
                            nc.vector.tensor_copy(
                                out=stash[:, t * 128:(t + 1) * 128],
                                in_=pst[:])
                            nc.sync.dma_start(
                                h_own[:][t * 128: t * 128 + rows, :],
                                stash[:rows, t * 128:(t + 1) * 128])
                            # fire AllGathers as soon as their rows are done
                            while ag_next[0] < 4 and \
                                    (t + 1) * 128 >= (ag_next[0] + 1) * cfg.QROWS:
                                s = ag_next[0]
                                ag_next[0] += 1
                                nc.gpsimd.collective_compute(
                                    "AllGather", OP.bypass,
                                    replica_groups=[list(range(NCORES))],
                                    ins=[h_own[:][s * cfg.QROWS:
                                                  (s + 1) * cfg.QROWS, :]],
                                    outs=[h_ag[s][:]])
                        else:
                            ps1 = mmps.tile([128, 128], f32, tag="mm")
                            nc.tensor.matmul(out=ps1[:], lhsT=w3[:], rhs=aggT[:],
                                             start=True, stop=True)
                            h3 = wp.tile([128, 128], f32, tag="h1")
                            nc.scalar.activation(out=h3[:], in_=ps1[:],
                                                 func=AT.Relu, bias=b3[:])
                            ps2 = mmps.tile([128, 128], f32, tag="mm")
                            nc.tensor.matmul(out=ps2[:CLS, :128],
                                             lhsT=w4[:], rhs=h3[:],
                                             start=True, stop=True)
                            c4 = sp.tile([CLS, 128], f32, tag="c4")
                            nc.vector.tensor_tensor(
                                out=c4[:], in0=ps2[:CLS, :128],
                                in1=b4[:].broadcast_to([CLS, 128]),
                                op=OP.add)
                            psf = mmps.tile([128, 128], f32, tag="mm")
                            nc.tensor.transpose(out=psf[:128, :CLS], in_=c4[:],
                                                identity=ident[:CLS, :CLS])
                            mx = sp.tile([128, 1], f32, tag="mx")
                            nc.vector.tensor_reduce(
                                out=mx[:], in_=psf[:128, :CLS],
                                axis=mybir.AxisListType.X, op=OP.max)
                            tsh = sp.tile([128, CLS], f32, tag="tsh")
                            nc.vector.tensor_tensor(
                                out=tsh[:], in0=psf[:128, :CLS],
                                in1=mx[:].broadcast_to([128, CLS]),
                                op=OP.subtract)
                            esum = sp.tile([128, 1], f32, tag="esum")
                            edum = sp.tile([128, CLS], f32, tag="edum")
                            nc.scalar.activation(out=edum[:], in_=tsh[:],
                                                 func=AT.Exp, accum_out=esum[:])
                            lse = sp.tile([128, 1], f32, tag="lse")
                            nc.scalar.activation(out=lse[:], in_=esum[:],
                                                 func=AT.Ln)
                            osb = sp.tile([128, CLS], f32, tag="osb")
                            nc.vector.tensor_tensor(
                                out=osb[:], in0=tsh[:],
                                in1=lse[:].broadcast_to([128, CLS]),
                                op=OP.subtract)
                            nc.sync.dma_start(
                                out_t.ap()[t * 128: t * 128 + rows, :],
                                osb[:rows, :])

            l1b = int(os.environ.get("GIN_L1_BATCHES", "0"))
            no_ag = bool(os.environ.get("GIN_NO_AG"))
            if l1b:
                layer([x.ap() for x in xq], is_first=True,
                      batch_limit=l1b, skip_mlp=True)
            else:
                layer([x.ap() for x in xq], is_first=True)
                if no_ag:
                    layer([x.ap() for x in xq], is_first=False)
                else:
                    layer([h[:] for h in h_ag], is_first=False)

    nc.compile()
    return nc


def _run(inputs, cfg):
    from concourse.bass_utils import run_bass_kernel_spmd

    x = np.asarray(inputs["x"], np.float32)
    edge_index = np.asarray(inputs["edge_index"])
    eps1 = float(np.asarray(inputs["eps1"]))
    eps2 = float(np.asarray(inputs["eps2"]))

    sched, gidx_all, dstloc_all = _prep_graph(edge_index, cfg)
    xbf = x.astype(BF16)
    xqs = _perm_rows(xbf, cfg)

    nc = _build_nc(cfg, sched, eps1, eps2)

    iota_np = np.tile(np.arange(W, dtype=np.float32),
                      (128, 1)).astype(BF16)
    identb_np = np.eye(128, dtype=np.float32).astype(BF16)
    ident_np = np.eye(128, dtype=np.float32)
    base = {
        "w1": np.asarray(inputs["w1"], np.float32),
        "w2": np.asarray(inputs["w2"], np.float32),
        "w3": np.asarray(inputs["w3"], np.float32),
        "w4": np.asarray(inputs["w4"], np.float32),
        "b1": np.asarray(inputs["b1"], np.float32).reshape(-1, 1),
        "b2": np.asarray(inputs["b2"], np.float32).reshape(-1, 1),
        "b3": np.asarray(inputs["b3"], np.float32).reshape(-1, 1),
        "b4": np.asarray(inputs["b4"], np.float32).reshape(-1, 1),
        "iota": iota_np,
        "identb": identb_np,
        "ident": ident_np,
    }
    for q in range(4):
        base[f"xq{q}"] = np.ascontiguousarray(xqs[q])

    in_maps = []
    for c in range(NCORES):
        m = dict(base)
        xo = np.zeros((cfg.NT * 128, cfg.F), BF16)
        xo[:cfg.NPC] = xbf[c * cfg.NPC:(c + 1) * cfg.NPC]
        m["xown"] = xo
        m["gidx"] = gidx_all[c]
        m["dstloc"] = dstloc_all[c]
        in_maps.append(m)

    if RUN_HOOK is not None:
        res = RUN_HOOK(nc, in_maps, core_ids=list(range(NCORES)))
    else:
        res = run_bass_kernel_spmd(nc, in_maps, core_ids=list(range(NCORES)))
    global LAST_RES
    LAST_RES = res
    out = np.concatenate([r["out"] for r in res.results], axis=0)
    return out.astype(np.float32)


LAST_RES = None
RUN_HOOK = None


def kernel(**inputs):
    return _run(inputs, FULL)


# revision 6
# speedup vs baseline: 1.2716x; 1.2716x over previous
"""GIN 2-layer message-passing network on 8 Trainium2 NeuronCores.

v2 strategy (dst-partitioned, per the sharding hint):
  - Nodes split into 8 chunks of N/8; core c owns chunk c and all edges
    whose destination lands in it.
  - Edges are grouped by (64-wide dst group, source quarter) and padded to
    128-slot chunks (cross-core max so one SPMD NEFF serves all cores).
    Gathered source rows (bf16) are scatter-added on the tensor engine
    via one-hot matmuls:  agg[feat, d] += G[e, feat].T @ OH[e, d<64>].
    64-wide one-hot columns halve the DVE is_equal work vs 128-wide.
  - The GIN self term (1+eps)*h_i is realized by initializing each dst
    tile's PSUM accumulation with an identity matmul of the tile's own
    feature rows (no self-edges in the gather).
  - MLP runs in transposed land ([feat, nodes]); per 128-dst tile:
    h = relu(w.T @ aggT + b) via PE matmul + ACT relu-with-bias.
  - Between layers, per-core h chunks are exchanged with 4 AllGathers
    (one per quarter, fired as soon as their rows are done) into Shared
    DRAM tensors so layer-2 gathers can use int16 indices.
  - log_softmax of the final [40, nodes] tile after a PE transpose.

All per-core variability lives in the data (gather indices / dst-local
arrays, padded to a per-group max across cores) so a single SPMD NEFF
serves all 8 cores.
"""

import os
import sys

sys.path.insert(0, "/opt/trn_rl_repo")
sys.path.insert(0, "/opt/trn_rl_repo/concourse")
os.environ.setdefault("TRN_TYPE", "TRN2")

import numpy as np
import ml_dtypes

BF16 = ml_dtypes.bfloat16

NCORES = 8
W = 64                    # dst-group width (one-hot column count)


class Cfg:
    def __init__(self, n, feat, hid, cls, tiles_per_batch=5):
        assert n % (NCORES * 4) == 0
        self.N = n
        self.F = feat
        self.H = hid
        self.CLS = cls
        self.NPC = n // NCORES          # nodes per core
        self.QROWS = self.NPC // 4      # rows per quarter per core
        self.SRCROWS = self.QROWS * NCORES  # rows per gather-source tensor
        self.NT = -(-self.NPC // 128)   # 128-dst tiles per core
        self.NG = self.NT * 2           # 64-wide dst groups per core
        self.last_rows = self.NPC - (self.NT - 1) * 128
        self.B = tiles_per_batch


FULL = Cfg(100000, 128, 128, 40, tiles_per_batch=5)


def _prep_graph(edge_index, cfg):
    """Host-side sharding. Groups edges by (dst 64-group, src quarter),
    pads each group to 128-slot chunks with the max count across cores.

    Returns (schedule, per-core gidx wraps, per-core dstloc arrays)."""
    N, NPC, QROWS, NG = cfg.N, cfg.NPC, cfg.QROWS, cfg.NG
    src = np.asarray(edge_index[0], dtype=np.int64)
    dst = np.asarray(edge_index[1], dtype=np.int64)

    core = dst // NPC
    per_core = []
    counts = np.zeros((NCORES, NG * 4), np.int64)
    for c in range(NCORES):
        m = core == c
        s = src[m]
        dloc = dst[m] - c * NPC
        g = dloc // W
        q = (s % NPC) // QROWS
        gid = g * 4 + q
        gidxv = (s // NPC) * QROWS + (s % QROWS)
        dstin = dloc % W
        counts[c] = np.bincount(gid, minlength=NG * 4)
        per_core.append((gid, gidxv.astype(np.int32), dstin.astype(np.int32)))

    cmax = counts.max(axis=0)
    C = -(-cmax // 128)                 # chunks per (group, quarter)
    slots = C * 128
    B = cfg.B
    batches = [list(range(b, min(b + B, cfg.NT))) for b in range(0, cfg.NT, B)]
    off = 0
    slot_off = np.zeros(NG * 4, np.int64)
    call_slots, call_off = [], []
    for tiles in batches:
        cs, co = [], []
        groups = [2 * t + h for t in tiles for h in range(2)]
        for q in range(4):
            co.append(off)
            s0 = off
            for g in groups:
                slot_off[g * 4 + q] = off
                off += slots[g * 4 + q]
            cs.append(off - s0)
        call_slots.append(cs)
        call_off.append(co)
    tot = off
    assert tot % 128 == 0

    gidx_all, dstloc_all = [], []
    for c in range(NCORES):
        gid, gidxv, dstin = per_core[c]
        order = np.argsort(gid, kind="stable")
        gs = gid[order]
        cnt = counts[c]
        starts = np.zeros(NG * 4, np.int64)
        np.cumsum(cnt[:-1], out=starts[1:])
        rank = np.arange(len(gs)) - starts[gs]
        slot = slot_off[gs] + rank
        gflat = np.zeros(tot, np.int16)
        dflat = np.full(tot, 200.0, np.float32)
        gflat[slot] = gidxv[order].astype(np.int16)
        dflat[slot] = dstin[order]
        # wrap for dma_gather: [p, col] = gflat[col*16 + p%16], replicated x8
        gwr = np.tile(gflat.reshape(tot // 16, 16).T, (8, 1)).copy()
        dloc = dflat.reshape(tot // 128, 128).T.astype(BF16).copy()
        gidx_all.append(gwr)
        dstloc_all.append(dloc)

    sched = dict(C=C, slots=slots, batches=batches, call_slots=call_slots,
                 call_off=call_off, slot_off=slot_off, tot=tot)
    return sched, gidx_all, dstloc_all


def _perm_rows(x, cfg):
    """x [N, F] -> 4 arrays [SRCROWS, F]; source s holds global row
    g = r*NPC + s*QROWS + u at position r*QROWS + u."""
    N, NPC, QROWS = cfg.N, cfg.NPC, cfg.QROWS
    g = np.arange(N)
    s = (g % NPC) // QROWS
    pos = (g // NPC) * QROWS + (g % QROWS)
    out = []
    for si in range(4):
        m = s == si
        a = np.empty((cfg.SRCROWS, x.shape[1]), x.dtype)
        a[pos[m]] = x[m]
        out.append(a)
    return out


def _build_nc(cfg, sched, eps1, eps2):
    from concourse import mybir
    import concourse.bacc as bacc
    import concourse.tile as tile

    F, H, CLS, NT, NPC = cfg.F, cfg.H, cfg.CLS, cfg.NT, cfg.NPC
    C = sched["C"]
    batches = sched["batches"]
    call_slots = sched["call_slots"]
    call_off = sched["call_off"]
    tot = sched["tot"]
    f32 = mybir.dt.float32
    bf16 = mybir.dt.bfloat16
    AT = mybir.ActivationFunctionType
    OP = mybir.AluOpType

    assert eps1 == 0.0 and eps2 == 0.0, "nonzero eps not implemented"

    nc = bacc.Bacc("TRN2", target_bir_lowering=False, debug=False,
                   num_devices=NCORES, num_swdge_queues=4)

    xq = [nc.dram_tensor(f"xq{q}", [cfg.SRCROWS, F], bf16, kind="ExternalInput")
          for q in range(4)]
    xown_t = nc.dram_tensor("xown", [NT * 128, F], bf16, kind="ExternalInput")
    w1_t = nc.dram_tensor("w1", [F, H], f32, kind="ExternalInput")
    w2_t = nc.dram_tensor("w2", [H, H], f32, kind="ExternalInput")
    w3_t = nc.dram_tensor("w3", [H, H], f32, kind="ExternalInput")
    w4_t = nc.dram_tensor("w4", [H, CLS], f32, kind="ExternalInput")
    b1_t = nc.dram_tensor("b1", [H, 1], f32, kind="ExternalInput")
    b2_t = nc.dram_tensor("b2", [H, 1], f32, kind="ExternalInput")
    b3_t = nc.dram_tensor("b3", [H, 1], f32, kind="ExternalInput")
    b4_t = nc.dram_tensor("b4", [CLS, 1], f32, kind="ExternalInput")
    iota_t = nc.dram_tensor("iota", [128, W], bf16, kind="ExternalInput")
    identb_t = nc.dram_tensor("identb", [128, 128], bf16, kind="ExternalInput")
    ident_t = nc.dram_tensor("ident", [128, 128], f32, kind="ExternalInput")
    gidx_t = nc.dram_tensor("gidx", [128, tot // 16], mybir.dt.int16,
                            kind="ExternalInput")
    dstloc_t = nc.dram_tensor("dstloc", [128, tot // 128], bf16,
                              kind="ExternalInput")
    out_t = nc.dram_tensor("out", [NPC, CLS], f32, kind="ExternalOutput")

    maxS = max(max(cs) for cs in call_slots)

    with tile.TileContext(nc) as tc:
        with tc.tile_pool(name="const", bufs=1) as cp, \
             tc.tile_pool(name="gp", bufs=6) as gp, \
             tc.tile_pool(name="ohp", bufs=6) as ohp, \
             tc.tile_pool(name="idxp", bufs=3) as idxp, \
             tc.tile_pool(name="dlp", bufs=3) as dlp, \
             tc.tile_pool(name="ownp", bufs=6) as ownp, \
             tc.tile_pool(name="work", bufs=4) as wp, \
             tc.tile_pool(name="small", bufs=6) as sp, \
             tc.tile_pool(name="aggps", bufs=4, space="PSUM") as aggps, \
             tc.tile_pool(name="mmps", bufs=4, space="PSUM") as mmps, \
             tc.tile_pool(name="dram", bufs=1, space="DRAM") as dp, \
             tc.tile_pool(name="shdram", bufs=1, space="DRAM") as shp:

            w1 = cp.tile([F, H], f32); nc.sync.dma_start(w1[:], w1_t.ap())
            w2 = cp.tile([H, H], f32); nc.sync.dma_start(w2[:], w2_t.ap())
            w3 = cp.tile([H, H], f32); nc.sync.dma_start(w3[:], w3_t.ap())
            w4 = cp.tile([H, CLS], f32); nc.sync.dma_start(w4[:], w4_t.ap())
            b1 = cp.tile([H, 1], f32); nc.sync.dma_start(b1[:], b1_t.ap())
            b2 = cp.tile([H, 1], f32); nc.sync.dma_start(b2[:], b2_t.ap())
            b3 = cp.tile([H, 1], f32); nc.sync.dma_start(b3[:], b3_t.ap())
            b4 = cp.tile([CLS, 1], f32); nc.sync.dma_start(b4[:], b4_t.ap())
            iota = cp.tile([128, W], bf16); nc.sync.dma_start(iota[:], iota_t.ap())
            identb = cp.tile([128, 128], bf16)
            nc.sync.dma_start(identb[:], identb_t.ap())
            ident = cp.tile([128, 128], f32); nc.sync.dma_start(ident[:], ident_t.ap())
            stash = cp.tile([128, NT * 128], bf16)

            h_own = dp.tile([NPC, H], bf16)
            h_ag = [shp.tile([cfg.SRCROWS, H], bf16, addr_space="Shared",
                             name=f"h_ag{s}")
                    for s in range(4)]

            def layer(sources, is_first, batch_limit=None, skip_mlp=False):
                ag_next = [0]

                for b, tiles in enumerate(batches):
                    if batch_limit is not None and b >= batch_limit:
                        break
                    base = call_off[b][0]
                    bslots = call_off[b][3] + call_slots[b][3] - base
                    # batched index loads (gidx always; dstloc both layers)
                    gi = idxp.tile([128, maxS * 4 // 16], mybir.dt.int16,
                                   tag="idx")
                    nc.sync.dma_start(
                        gi[:, : bslots // 16],
                        gidx_t.ap()[:, base // 16:(base + bslots) // 16])
                    dl = dlp.tile([128, maxS * 4 // 128], bf16, tag="dl")
                    nc.sync.dma_start(
                        dl[:, : bslots // 128],
                        dstloc_t.ap()[:, base // 128:(base + bslots) // 128])

                    G, OH = [], []
                    for q in range(4):
                        S = call_slots[b][q]
                        if S == 0:
                            G.append(None); OH.append(None)
                            continue
                        o = call_off[b][q]
                        g = gp.tile([128, maxS // 128, 128], bf16, tag="g")
                        nc.gpsimd.dma_gather(
                            g[:, : S // 128, :], sources[q],
                            gi[:, (o - base) // 16:(o - base + S) // 16],
                            S, S, F, single_packet=False, queue_num=q)
                        oh = ohp.tile([128, maxS // 128, W], bf16, tag="oh")
                        nc.vector.tensor_tensor(
                            out=oh[:, : S // 128, :],
                            in0=iota[:].unsqueeze(1).broadcast_to(
                                [128, S // 128, W]),
                            in1=dl[:, (o - base) // 128:(o - base + S) // 128]
                                .unsqueeze(2).broadcast_to([128, S // 128, W]),
                            op=OP.is_equal)
                        G.append(g); OH.append(oh)

                    pos = [0, 0, 0, 0]
                    for t in tiles:
                        # chunks per (half, quarter) for this tile
                        nch = [[int(C[(2 * t + h) * 4 + q]) for q in range(4)]
                               for h in range(2)]
                        tot_ch = sum(sum(r) for r in nch)
                        agg = aggps.tile([128, 128], f32, tag="agg")
                        own = None
                        if is_first:
                            own = ownp.tile([128, 128], bf16, tag="own")
                            nc.sync.dma_start(
                                own[:], xown_t.ap()[t * 128:(t + 1) * 128, :])
                            own_ap = own[:]
                        else:
                            own_ap = stash[:, t * 128:(t + 1) * 128]
                        # self term: agg = own.T (identity matmul opens group)
                        nc.tensor.matmul(out=agg[:], lhsT=own_ap, rhs=identb[:],
                                         start=True, stop=(tot_ch == 0))
                        k = 0
                        for h in range(2):
                            for q in range(4):
                                if nch[h][q] == 0:
                                    continue
                                for j in range(nch[h][q]):
                                    col = pos[q] + (0 if h == 0 else nch[0][q]) + j
                                    k += 1
                                    nc.tensor.matmul(
                                        out=agg[:, h * W:(h + 1) * W],
                                        lhsT=G[q][:, col, :],
                                        rhs=OH[q][:, col, :],
                                        start=False,
                                        stop=(k == tot_ch),
                                        skip_group_check=True)
                        for q in range(4):
                            pos[q] += nch[0][q] + nch[1][q]
                        rows = 128 if t < NT - 1 else cfg.last_rows

                        if skip_mlp:
                            continue
                        aggT = wp.tile([128, 128], f32, tag="aggT")
                        nc.scalar.activation(out=aggT[:], in_=agg[:], func=AT.Copy)

                        if is_first:
                            ps1 = mmps.tile([128, 128], f32, tag="mm")
                            nc.tensor.matmul(out=ps1[:], lhsT=w1[:], rhs=aggT[:],
                                             start=True, stop=True)
                            h1 = wp.tile([128, 128], f32, tag="h1")
                            nc.scalar.activation(out=h1[:], in_=ps1[:],
                                                 func=AT.Relu, bias=b1[:])
                            ps2 = mmps.tile([128, 128], f32, tag="mm")
                            nc.tensor.matmul(out=ps2[:], lhsT=w2[:], rhs=h1[:],
                                             start=True, stop=True)
                            h2 = wp.tile([128, 128], f32, tag="h2")
                            nc.scalar.activation(out=h2[:], in_=ps2[:],
                                                 func=AT.Relu, bias=b2[:])
                            # transpose back to [nodes, feat], cast bf16, stash
                            pst = mmps.tile([128, 128], f32, tag="mm")
                            nc.tensor.transpose(out=pst[:], in_=h2[:],
                                                identity=ident[:])
                            nc.vector.tensor_copy(
                                out=stash[:, t * 128:(t + 1) * 128],
                                in_=pst[:])
                            nc.sync.dma_start(
                                h_own[:][t * 128: t * 128 + rows, :],
                                stash[:rows, t * 128:(t + 1) * 128])
                            # fire AllGathers as soon as their rows are done
                            while ag_next[0] < 4 and \
                                    (t + 1) * 128 >= (ag_next[0] + 1) * cfg.QROWS:
                                s = ag_next[0]
                                ag_next[0] += 1
                                nc.gpsimd.collective_compute(
                                    "AllGather", OP.bypass,
                                    replica_groups=[list(range(NCORES))],
                                    ins=[h_own[:][s * cfg.QROWS:
                                                  (s + 1) * cfg.QROWS, :]],
                                    outs=[h_ag[s][:]])
                        else:
                            ps1 = mmps.tile([128, 128], f32, tag="mm")
                            nc.tensor.matmul(out=ps1[:], lhsT=w3[:], rhs=aggT[:],
                                             start=True, stop=True)
                            h3 = wp.tile([128, 128], f32, tag="h1")
                            nc.scalar.activation(out=h3[:], in_=ps1[:],
                                                 func=AT.Relu, bias=b3[:])
                            ps2 = mmps.tile([128, 128], f32, tag="mm")
                            nc.tensor.matmul(out=ps2[:CLS, :128],
                                             lhsT=w4[:], rhs=h3[:],
                                             start=True, stop=True)
                            c4 = sp.tile([CLS, 128], f32, tag="c4")
                            nc.vector.tensor_tensor(
                                out=c4[:], in0=ps2[:CLS, :128],
                                in1=b4[:].broadcast_to([CLS, 128]),
                                op=OP.add)
                            psf = mmps.tile([128, 128], f32, tag="mm")
                            nc.tensor.transpose(out=psf[:128, :CLS], in_=c4[:],
                                                identity=ident[:CLS, :CLS])
                            mx = sp.tile([128, 1], f32, tag="mx")
                            nc.vector.tensor_reduce(
                                out=mx[:], in_=psf[:128, :CLS],
                                axis=mybir.AxisListType.X, op=OP.max)
                            tsh = sp.tile([128, CLS], f32, tag="tsh")
                            nc.vector.tensor_tensor(
                                out=tsh[:], in0=psf[:128, :CLS],
                                in1=mx[:].broadcast_to([128, CLS]),
                                op=OP.subtract)
                            esum = sp.tile([128, 1], f32, tag="esum")
                            edum = sp.tile([128, CLS], f32, tag="edum")
                            nc.scalar.activation(out=edum[:], in_=tsh[:],
                                                 func=AT.Exp, accum_out=esum[:])
                            lse = sp.tile([128, 1], f32, tag="lse")
                            nc.scalar.activation(out=lse[:], in_=esum[:],
                                                 func=AT.Ln)
                            osb = sp.tile([128, CLS], f32, tag="osb")
                            nc.vector.tensor_tensor(
                                out=osb[:], in0=tsh[:],
                                in1=lse[:].broadcast_to([128, CLS]),
                                op=OP.subtract)
                            nc.sync.dma_start(
                                out_t.ap()[t * 128: t * 128 + rows, :],
                                osb[:rows, :])

            l1b = int(os.environ.get("GIN_L1_BATCHES", "0"))
            no_ag = bool(os.environ.get("GIN_NO_AG"))
            if l1b:
                layer([x.ap() for x in xq], is_first=True,
                      batch_limit=l1b, skip_mlp=True)
            else:
                layer([x.ap() for x in xq], is_first=True)
                if no_ag:
                    layer([x.ap() for x in xq], is_first=False)
                else:
                    layer([h[:] for h in h_ag], is_first=False)

    nc.compile()
    return nc


def _run(inputs, cfg):
    from concourse.bass_utils import run_bass_kernel_spmd

    x = np.asarray(inputs["x"], np.float32)
    edge_index = np.asarray(inputs["edge_index"])
    eps1 = float(np.asarray(inputs["eps1"]))
    eps2 = float(np.asarray(inputs["eps2"]))

    sched, gidx_all, dstloc_all = _prep_graph(edge_index, cfg)
    xbf = x.astype(BF16)
    xqs = _perm_rows(xbf, cfg)

    nc = _build_nc(cfg, sched, eps1, eps2)

    iota_np = np.tile(np.arange(W, dtype=np.float32),
                      (128, 1)).astype(BF16)
    identb_np = np.eye(128, dtype=np.float32).astype(BF16)
    ident_np = np.eye(128, dtype=np.float32)
    base = {
        "w1": np.asarray(inputs["w1"], np.float32),
        "w2": np.asarray(inputs["w2"], np.float32),
        "w3": np.asarray(inputs["w3"], np.float32),
        "w4": np.asarray(inputs["w4"], np.float32),
        "b1": np.asarray(inputs["b1"], np.float32).reshape(-1, 1),
        "b2": np.asarray(inputs["b2"], np.float32).reshape(-1, 1),
        "b3": np.asarray(inputs["b3"], np.float32).reshape(-1, 1),
        "b4": np.asarray(inputs["b4"], np.float32).reshape(-1, 1),
        "iota": iota_np,
        "identb": identb_np,
        "ident": ident_np,
    }
    for q in range(4):
        base[f"xq{q}"] = np.ascontiguousarray(xqs[q])

    in_maps = []
    for c in range(NCORES):
        m = dict(base)
        xo = np.zeros((cfg.NT * 128, cfg.F), BF16)
        xo[:cfg.NPC] = xbf[c * cfg.NPC:(c + 1) * cfg.NPC]
        m["xown"] = xo
        m["gidx"] = gidx_all[c]
        m["dstloc"] = dstloc_all[c]
        in_maps.append(m)

    if RUN_HOOK is not None:
        res = RUN_HOOK(nc, in_maps, core_ids=list(range(NCORES)))
    else:
        res = run_bass_kernel_spmd(nc, in_maps, core_ids=list(range(NCORES)))
    global LAST_RES
    LAST_RES = res
    out = np.concatenate([r["out"] for r in res.results], axis=0)
    return out.astype(np.float32)


LAST_RES = None
RUN_HOOK = None


def kernel(**inputs):
    return _run(inputs, FULL)
